# revision 5
# baseline (speedup 1.0000x reference)
"""Trainium2 Bass kernel for nn_Disp_61125974557155 (V1: all-bf16, R-stationary stats).

Computes: trilinear upsample of a cost volume [B,1,48,64,128] ->
[B,193,256,512] (align_corners=False, edge-replicated), softmin over
disparity, disparity regression -> [B,256,512].

Design (per core; 8 cores = 2 batches x 4 H'-quarters):
  - Host: edge-pad x (replicate), slice the core's H-halo shard, stack a
    copy shifted by one h-row on partitions 50..99, cast bf16 (sharding /
    layout only).
  - DVE: W-axis 4x lerp at low resolution -> xsw [100, 19, 4, 128] bf16.
  - PE: D-expansion (48->193 banded lerp matrix, bf16) with the H-axis 4x
    lerp FOLDED into the matmul via the dup-shifted operand halves:
    vol = A2r^T @ xsw_row -> PSUM [d'-chunk, 512] tiles.
  - ACT: e = exp(-vol) (PSUM -> SBUF, bf16), FD-1024 tiles.
  - PE: stats with rmat STATIONARY ([dn, 2] = {1, d}) and e MOVING:
    out [2, 512] per (t, r) accumulated over the two d'-chunks. This
    streams e through the array at 128 elem/cycle @ 2.4 GHz instead of
    paying a 128-col LDWEIGHTS (1.2 GHz) per 2 streamed columns like the
    flipped layout does.
  - DVE: copy stat rows [2, 2048] PSUM->SBUF per t; DMA scatters them
    SBUF->SBUF into pixel-major [64, 512] S0/S1; DVE recip+mul finalize;
    DMA out.
"""

import numpy as np
from contextlib import ExitStack

import concourse.bass as bass
import concourse.bacc as bacc
import concourse.tile as tile
from concourse import mybir
from concourse.bass_utils import run_bass_kernel_spmd

F32 = mybir.dt.float32
BF16 = mybir.dt.bfloat16

MAXDISP = 192
DP = MAXDISP + 1      # 193 disparities
KD = 48               # low-res D
KP = KD + 2           # padded k' (edge-replicated)
NCORES = 8
WH = (0.625, 0.875, 0.125, 0.375)   # H lerp fracs per r = h' % 4
CHUNKS = ((0, 128), (128, 65))      # d' chunk (offset, size)
NROW = 19                            # h-rows in dup-packed shard
ROW_GROUPS = ((0, 2), (2, 2), (4, 4), (8, 4), (12, 4), (16, 1))


def _build_ad() -> np.ndarray:
    """A_D [193, 50]: D-axis linear upsample matrix on padded k' = k+1."""
    ad = np.zeros((DP, KP), dtype=np.float64)
    for dp in range(DP):
        i = (dp + 0.5) * KD / DP - 0.5
        fl = int(np.floor(i))
        fr = i - fl
        ad[dp, fl + 1] += 1.0 - fr
        ad[dp, fl + 2] += fr
    return ad


def _build_consts():
    ad = _build_ad()                      # [193, 50]
    amat = np.zeros((2 * KP, 4, DP), dtype=np.float64)
    for r in range(4):
        amat[:KP, r, :] = (1.0 - WH[r]) * ad.T
        amat[KP:, r, :] = WH[r] * ad.T
    rmat = np.zeros((128, 4), dtype=np.float64)
    rmat[:, 0] = 1.0
    rmat[:, 1] = np.arange(128)
    rmat[: DP - 128, 2] = 1.0
    rmat[: DP - 128, 3] = np.arange(128, DP)
    bf = mybir.dt.np(BF16)
    return (
        np.ascontiguousarray(amat.reshape(2 * KP, 4 * DP)).astype(bf),
        rmat.astype(np.float32).astype(bf),
    )


def _build_nc() -> bass.Bass:
    nc = bacc.Bacc()
    xsd = nc.declare_dram_parameter("xsd", [2 * KP, NROW * 130], BF16, isOutput=False)
    amat = nc.declare_dram_parameter("amat", [2 * KP, 4 * DP], BF16, isOutput=False)
    rmat = nc.declare_dram_parameter("rmat", [128, 4], BF16, isOutput=False)
    outp = nc.declare_dram_parameter("out", [64, 512], F32, isOutput=True)

    xsd_v = xsd.rearrange("p (h w) -> p h w", h=NROW)
    amat_v = amat.rearrange("p (r d) -> p r d", r=4)

    mult = mybir.AluOpType.mult
    add = mybir.AluOpType.add
    exp_fn = mybir.ActivationFunctionType.Exp

    with ExitStack() as ctx:
        tc = ctx.enter_context(tile.TileContext(nc))
        singles = ctx.enter_context(tc.tile_pool(name="singles", bufs=1))
        tmp_pool = ctx.enter_context(tc.tile_pool(name="tmp", bufs=2))
        epool = ctx.enter_context(tc.tile_pool(name="epool", bufs=6))
        dpool = ctx.enter_context(tc.tile_pool(name="dpool", bufs=2))
        fin = ctx.enter_context(tc.tile_pool(name="fin", bufs=1))
        pvol = ctx.enter_context(tc.tile_pool(name="pvol", bufs=2, space="PSUM"))
        pstat = ctx.enter_context(tc.tile_pool(name="pstat", bufs=1, space="PSUM"))

        # ---- input loads: xsd first (gates the lerp chain) on the sync
        # HWDGE queue; constants go through gpsimd SWDGE in parallel ----
        s_xsd = []
        for g, (g0, gn) in enumerate(ROW_GROUPS):
            t_x = singles.tile([2 * KP, gn, 130], BF16, tag=f"xsd{g}")
            nc.sync.dma_start(out=t_x, in_=xsd_v[:, g0 : g0 + gn, :])
            s_xsd.append(t_x)
        s_am = {}
        for ci, (d0, dn) in enumerate(CHUNKS):
            for r in range(4):
                t_a = singles.tile([2 * KP, dn], BF16, tag=f"am{ci}{r}")
                nc.gpsimd.dma_start(out=t_a, in_=amat_v[:, r, d0 : d0 + dn])
                s_am[(ci, r)] = t_a
        s_rm = singles.tile([128, 4], BF16, tag="rm")
        nc.gpsimd.dma_start(out=s_rm, in_=rmat[:, :])

        # ---- W-axis 4x lerp at low res, rw-major planes (bf16, DVE 2x) ----
        # xsw[p, h, rw, s] = lerp; shared difference d[s] = xs[s] - xs[s+1]:
        #   rw0 = xs[s+1] + 0.375*d[s]    rw1 = xs[s+1] + 0.125*d[s]
        #   rw2 = xs[s+2] + 0.875*d[s+1]  rw3 = xs[s+2] + 0.625*d[s+1]
        s_xsw = []
        for g, (g0, gn) in enumerate(ROW_GROUPS):
            t_w = singles.tile([2 * KP, gn, 4, 128], BF16, tag=f"xsw{g}")
            t_d = tmp_pool.tile([2 * KP, gn, 129], BF16, tag="wld")
            nc.vector.tensor_sub(
                t_d, s_xsd[g][:, :, 0:129], s_xsd[g][:, :, 1:130]
            )
            for rw, (coef, dc, hc) in enumerate(
                ((0.375, 0, 1), (0.125, 0, 1), (0.875, 1, 2), (0.625, 1, 2))
            ):
                nc.vector.scalar_tensor_tensor(
                    out=t_w[:, :, rw, :],
                    in0=t_d[:, :, dc : dc + 128],
                    scalar=coef,
                    in1=s_xsd[g][:, :, hc : hc + 128],
                    op0=mult,
                    op1=add,
                )
            s_xsw.append(t_w)

        def xsw_row(l: int) -> bass.AP:
            for g, (g0, gn) in enumerate(ROW_GROUPS):
                if g0 <= l < g0 + gn:
                    return s_xsw[g][:, l - g0, :, :]
            raise IndexError(l)

        # ---- S0/S1 pixel-major accumulators (filled by per-t DMA scatter) ----
        s0mat = fin.tile([64, 512], F32, tag="s0mat")
        s1mat = fin.tile([64, 512], F32, tag="s1mat")

        # ---- main loop over coarse h-rows t; r = h' % 4, j = 4t + r ----
        for t in range(16):
            ps = pstat.tile([128, 4, 512], F32, tag="ps")
            for ci, (d0, dn) in enumerate(CHUNKS):
                for rp in range(2):
                    pv = pvol.tile([128, 2, 512], F32, tag="pv")
                    et = epool.tile([128, 2, 512], BF16, tag="e")
                    for u in range(2):
                        r = 2 * rp + u
                        l = t if r < 2 else t + 1
                        rhs = xsw_row(l).rearrange("p q s -> p (q s)")
                        nc.tensor.matmul(
                            pv[0:dn, u, :],
                            s_am[(ci, r)][:, :],
                            rhs,
                            start=True,
                            stop=True,
                        )
                    nc.scalar.activation(
                        et[0:dn, :, :], pv[0:dn, :, :], exp_fn, scale=-1.0
                    )
                    for u in range(2):
                        r = 2 * rp + u
                        nc.tensor.matmul(
                            ps[0:2, r, :],
                            s_rm[0:dn, 2 * ci : 2 * ci + 2],
                            et[0:dn, u, :],
                            start=(ci == 0),
                            stop=(ci == 1),
                            skip_group_check=True,
                        )
            # drain stats: PSUM -> SBUF (DVE), then DMA-scatter to pixel-major
            sd = dpool.tile([2, 4, 512], F32, tag="sd")
            nc.vector.tensor_copy(sd, ps[0:2, :, :])
            nc.gpsimd.dma_start(out=s0mat[4 * t : 4 * t + 4, :], in_=sd[0:1, :, :])
            nc.gpsimd.dma_start(out=s1mat[4 * t : 4 * t + 4, :], in_=sd[1:2, :, :])

        # ---- finalize: disp = S1 * recip(S0), partition-parallel ----
        # stat columns are in (rw, s) order (pixel w' = 4s + rw); un-permute
        # to w'-order during the multiply via a strided output AP.
        rec = fin.tile([64, 512], F32, tag="rec")
        om = fin.tile([64, 512], F32, tag="om")
        nc.vector.reciprocal(rec, s0mat)
        nc.vector.tensor_mul(
            om.rearrange("j (s q) -> j q s", q=4),
            s1mat.rearrange("j (q s) -> j q s", q=4),
            rec.rearrange("j (q s) -> j q s", q=4),
        )
        nc.sync.dma_start(out=outp[:, :], in_=om)

    nc.compile()
    return nc


_CACHE: dict = {}


def _shard_inputs(x: np.ndarray):
    """Edge-pad and slice per-core shards (layout + dtype cast only)."""
    xpad = np.pad(x[:, 0], ((0, 0), (1, 1), (1, 3), (1, 1)), mode="edge")
    amat, rmat = _build_consts()
    bf = mybir.dt.np(BF16)
    in_maps = []
    for c in range(NCORES):
        b, q = divmod(c, 4)
        xs = xpad[b][:, 16 * q : 16 * q + 20, :]          # [50, 20, 130]
        xsd = np.concatenate([xs[:, 0:19, :], xs[:, 1:20, :]], axis=0)
        xsd = np.ascontiguousarray(xsd.reshape(2 * KP, NROW * 130)).astype(bf)
        in_maps.append({"xsd": xsd, "amat": amat, "rmat": rmat})
    return in_maps


def kernel(x: np.ndarray, _trace: bool = False, _tmpdir=None):
    x = np.asarray(x, dtype=np.float32)
    assert x.shape == (2, 1, 48, 64, 128), x.shape
    if "nc" not in _CACHE:
        _CACHE["nc"] = _build_nc()
    nc = _CACHE["nc"]
    in_maps = _shard_inputs(x)
    res = run_bass_kernel_spmd(
        nc, in_maps, list(range(NCORES)), trace=_trace, tmpdir=_tmpdir
    )
    out = np.zeros((2, 256, 512), dtype=np.float32)
    for c in range(NCORES):
        b, q = divmod(c, 4)
        out[b, 64 * q : 64 * (q + 1), :] = res.results[c]["out"]
    if _trace:
        return out, res
    return out
